# revision 14
# baseline (speedup 1.0000x reference)
"""Trainium2 Bass kernel for nn_ChannelNonlinearSpectralBlock.

Math
----
Per pixel column x (C=256 channels), the reference computes
    u  = g(||x||) * x                      (log map, per-pixel scalar gate)
    u1 = f1(||u||) * u                     (Fourier gate 1)
    v0 = irfft(rfft(u1) * Hf)              (fixed linear map: circulant Wc)
    v1 = f2(||v0||) * v0                   (Fourier gate 2)
    y  = t(||v1||) * v1                    (exp map)
    out = alpha*y + beta*x
All per-pixel scalars commute through the linear map Wc, so
    out = A * (Wc @ x) + beta * x
with A = alpha * P1(r0^2) * Q2(P1(r0^2)^2 * q^2), r0^2 = ||x||^2 and (by
Parseval; |Hf|=1 except the real-projected DC/Nyquist bins)
    q^2 = r0^2 - d0*X0^2 - d1*X128^2,
    X0 = sum_c x[c], X128 = sum_c (-1)^c x[c].
P1, Q2 are fitted host-side as polynomials over the observed data range
(coefficients are runtime data via a constant vector).  When the exactly
computed host-side bound on the d0/d1 correction's effect on A is far below
tolerance, the X0/X128 stats are skipped entirely (use_x=False variant).

Implementation (per core, pure data parallel over pixels)
---------------------------------------------------------
Channel-major layout [128 chans x pixels].  ScalarE casts x to bf16 once;
VectorE/GpSimd square the bf16 copy; TensorE reduces r0^2 with a resident
ones-column bf16 stationary (PSUM rows packed at {0,32,64} offsets) and
computes the 256x256 circulant matmul in bf16 (2x stream rate vs f32r).
Stats evacuate via strided engine copies + one rearrange DMA per chunk.
The gate chain runs on VectorE with scalar_tensor_tensor fused Horner
steps (1 op per degree).  GpSimd broadcasts A across partitions; VectorE
applies A * (Wc x) out of PSUM; outputs stream back with 0.5 MB DMAs.
"""

import numpy as np

import concourse.bass as bass
import concourse.bacc as bacc
from concourse import library_config
import concourse.mybir as mybir
from concourse.tile import TileContext

F32 = mybir.dt.float32
F32R = mybir.dt.float32r
BF16 = mybir.dt.bfloat16

# Problem shape (hardcoded per contract)
B, C, H, W = 32, 256, 64, 64
HWPIX = H * W  # 4096
NCORES = 8
B_CORE = B // NCORES  # 4 images per core
NPIX = B_CORE * HWPIX  # 16384 pixels per core
HALF = NPIX // 2  # 8192
CHUNK = 2048
N_CHUNK = NPIX // CHUNK  # 8
CH_HALF = N_CHUNK // 2  # 4 chunks per half
SUB = 512
N_SUB_CH = CHUNK // SUB  # 4 subtiles per chunk

C_CURV = 0.001
L = 10
N_HARM = 16
EPS = 1e-6

MAXDEG = 20

# cvec layout: [P1 coefs (MAXDEG+1, highest first)][Q2 coefs][misc]
ID_P1 = 0
ID_Q2 = MAXDEG + 1
ID_S1 = 2 * (MAXDEG + 1)
ID_T1 = ID_S1 + 1
ID_S2 = ID_S1 + 2
ID_T2 = ID_S1 + 3
ID_ND0 = ID_S1 + 4
ID_ND1 = ID_S1 + 5
ID_YLO = ID_S1 + 6
ID_YHI = ID_S1 + 7
ID_ZLO = ID_S1 + 8
ID_ZHI = ID_S1 + 9
ID_BETA = ID_S1 + 10
CV = 64
assert ID_BETA < CV


def build_program(beta_nonzero: bool, use_x: bool, d1: int, d2: int) -> bass.Bass:
    nc = bacc.Bacc(None, target_bir_lowering=False)
    x = nc.declare_dram_parameter("x", [B_CORE, C, HWPIX], F32, isOutput=False)
    wmat = nc.declare_dram_parameter("wmat", [2, 2, 128, 128], BF16, isOutput=False)
    saltv = nc.declare_dram_parameter("saltv", [128, 32], BF16, isOutput=False)
    cvec = nc.declare_dram_parameter("cvec", [1, CV], F32, isOutput=False)
    out = nc.declare_dram_parameter("out", [B_CORE, C, HWPIX], F32, isOutput=True)

    # channel-block views of DRAM x / out: [cb, 128, b, hw]
    xv = x.rearrange("b (cb p) hw -> cb p b hw", cb=2)
    ov = out.rearrange("b (cb p) hw -> cb p b hw", cb=2)

    AT = mybir.AluOpType

    with TileContext(nc) as tc:
        with (
            tc.tile_pool(name="const", bufs=1) as const,
            tc.tile_pool(name="xf", bufs=2) as xfp,
            tc.tile_pool(name="xb", bufs=1) as xbp,
            tc.tile_pool(name="sq", bufs=2) as sqp,
            tc.tile_pool(name="stg", bufs=2) as stgp,
            tc.tile_pool(name="chain", bufs=1) as chp,
            tc.tile_pool(name="astage", bufs=1) as asp,
            tc.tile_pool(name="abp", bufs=3) as abp,
            tc.tile_pool(name="outp", bufs=3) as outp,
            tc.tile_pool(name="stps", bufs=1 if use_x else 2, space="PSUM") as stps,
            tc.tile_pool(name="wps", bufs=2, space="PSUM") as wps,
        ):
            nc.gpsimd.load_library(library_config.mlp)

            # ---- constants ----
            wm = [[None, None], [None, None]]
            for kb in range(2):
                for mb in range(2):
                    t = const.tile([128, 128], BF16, name=f"wm{kb}{mb}", tag=f"wm{kb}{mb}")
                    nc.sync.dma_start(out=t, in_=wmat[kb, mb])
                    wm[kb][mb] = t
            salt = const.tile([128, 32], BF16, tag="saltv")
            nc.sync.dma_start(out=salt, in_=saltv[:, :])
            cv = const.tile([128, CV], F32, tag="cv")
            nc.sync.dma_start(out=cv, in_=cvec[0:1, :].partition_broadcast(128))

            def cvs(i):  # per-partition scalar AP for constant i
                return cv[:, i : i + 1]

            # resident bf16 x (both channel blocks, all chunks)
            xbt = [[None] * N_CHUNK, [None] * N_CHUNK]
            for blk in range(2):
                for c_ in range(N_CHUNK):
                    xbt[blk][c_] = xbp.tile(
                        [128, CHUNK], BF16, name=f"xb{blk}_{c_}", tag=f"xb{blk}_{c_}"
                    )

            for half in range(2):
                chR = chp.tile([128, 64], F32, tag=f"chR{half}")
                if use_x:
                    chS0 = chp.tile([128, 64], F32, tag=f"chS0{half}")
                    chS1 = chp.tile([128, 64], F32, tag=f"chS1{half}")

                # ---------- phase 1: load, cast, square, stats ----------
                for t_ in range(CH_HALF):
                    c_ = half * CH_HALF + t_
                    b_ = (c_ * CHUNK) // HWPIX
                    hw0 = (c_ * CHUNK) % HWPIX
                    sq = [None, None]
                    tsum = None
                    for blk in range(2):
                        xf = xfp.tile([128, CHUNK], F32, name=f"xf{blk}", tag=f"xf{blk}")
                        nc.sync.dma_start(
                            out=xf, in_=xv[blk, :, b_, hw0 : hw0 + CHUNK]
                        )
                        # cast to bf16 (ScalarE)
                        nc.scalar.copy(xbt[blk][c_], xf)
                        # square the bf16 copy (VectorE blk0 / GpSimd blk1)
                        sq[blk] = sqp.tile(
                            [128, CHUNK], BF16, name=f"sq{blk}", tag=f"sq{blk}"
                        )
                        eng = nc.vector if blk == 0 else nc.gpsimd
                        eng.tensor_tensor(
                            sq[blk], xbt[blk][c_], xbt[blk][c_], AT.mult
                        )
                    if use_x:
                        tsum = sqp.tile([128, CHUNK], BF16, tag="tsum")
                        nc.gpsimd.tensor_tensor(
                            tsum, xbt[0][c_], xbt[1][c_], AT.add
                        )
                    # r0^2: subtile s4 row at PSUM partition 32*s4 (one bank)
                    stR = stps.tile([128, SUB], F32, tag="stR")
                    if use_x:
                        stX = stps.tile([128, SUB], F32, tag="stX")
                    for s4 in range(N_SUB_CH):
                        sl = slice(s4 * SUB, s4 * SUB + SUB)
                        rout = stR[32 * s4 : 32 * s4 + 32, :]
                        nc.tensor.matmul(
                            rout, salt, sq[0][:, sl],
                            start=True, stop=False, tile_position=(0, 32 * s4),
                        )
                        nc.tensor.matmul(
                            rout, salt, sq[1][:, sl],
                            start=False, stop=True, tile_position=(0, 32 * s4),
                        )
                        if use_x:
                            nc.tensor.matmul(
                                stX[32 * s4 : 32 * s4 + 32, :], salt,
                                tsum[:, sl],
                                start=True, stop=True,
                                tile_position=(0, 32 * s4),
                            )
                    # evacuate: one contiguous engine copy PSUM->SBUF, then a
                    # partition-strided gather DMA into the chain layout
                    rstg = stgp.tile([128, SUB], F32, tag="rstg")
                    nc.scalar.copy(rstg, stR)
                    p0 = 32 * t_
                    nc.sync.dma_start(
                        out=chR[p0 : p0 + 32, :], in_=rstg[0:128:32, :]
                    )
                    if use_x:
                        xstg = stgp.tile([128, SUB], F32, tag="xstg")
                        nc.scalar.copy(xstg, stX)
                        nc.sync.dma_start(
                            out=chS0[p0 : p0 + 32, :], in_=xstg[0:128:32, :]
                        )
                        for s4 in range(N_SUB_CH):
                            nc.sync.dma_start(
                                out=chS1[p0 + 8 * s4 : p0 + 8 * s4 + 8, :],
                                in_=xstg[
                                    32 * s4 + 1 : 32 * s4 + 2, :
                                ].rearrange("o (p f) -> o p f", p=8),
                            )

                # ---------- phase 2: gate chain on [128, 64] ----------
                yc = chp.tile([128, 64], F32, tag="yc")
                q2 = chp.tile([128, 64], F32, tag="q2")
                ut = chp.tile([128, 64], F32, tag="ut")
                acc = chp.tile([128, 64], F32, tag="acc")
                p1 = chp.tile([128, 64], F32, tag="p1")
                zt = chp.tile([128, 64], F32, tag="zt")
                acc2 = chp.tile([128, 64], F32, tag="acc2")
                At = chp.tile([128, 64], F32, tag="At")

                V = nc.vector
                if use_x:
                    t0 = chp.tile([128, 64], F32, tag="t0")
                    # q2 = chR + nd0*X0^2 + nd1*X128^2 (nd* negated host-side)
                    V.scalar_tensor_tensor(t0, chS0, cvs(ID_ND0), chS0, AT.mult, AT.mult)
                    V.scalar_tensor_tensor(q2, chS1, cvs(ID_ND1), chS1, AT.mult, AT.mult)
                    V.tensor_tensor(q2, q2, t0, AT.add)
                    V.tensor_tensor(q2, q2, chR, AT.add)

                # u = clamp(r0^2)*s1 + t1 in [-1,1]
                V.tensor_scalar(yc, chR, cvs(ID_YLO), cvs(ID_YHI), AT.max, AT.min)
                V.tensor_scalar(ut, yc, cvs(ID_S1), cvs(ID_T1), AT.mult, AT.add)
                # P1 modified Horner: b=c_D*u; b=(b+c_k)*u ...; p1=b+c_0
                V.tensor_scalar(acc, ut, cvs(ID_P1), None, AT.mult)
                for k in range(1, d1):
                    V.scalar_tensor_tensor(
                        acc, acc, cvs(ID_P1 + k), ut, AT.add, AT.mult
                    )
                V.tensor_scalar(p1, acc, cvs(ID_P1 + d1), None, AT.add)

                # z = p1^2 * q2 (q2 = clamped r0^2 when X-stats skipped)
                V.tensor_tensor(zt, p1, p1, AT.mult)
                V.tensor_tensor(zt, zt, q2 if use_x else yc, AT.mult)
                V.tensor_scalar(zt, zt, cvs(ID_ZLO), cvs(ID_ZHI), AT.max, AT.min)
                V.tensor_scalar(zt, zt, cvs(ID_S2), cvs(ID_T2), AT.mult, AT.add)
                # Q2 modified Horner (alpha folded into coefficients)
                V.tensor_scalar(acc2, zt, cvs(ID_Q2), None, AT.mult)
                for k in range(1, d2):
                    V.scalar_tensor_tensor(
                        acc2, acc2, cvs(ID_Q2 + k), zt, AT.add, AT.mult
                    )
                V.tensor_scalar(acc2, acc2, cvs(ID_Q2 + d2), None, AT.add)

                V.tensor_tensor(At, p1, acc2, AT.mult)

                ast = asp.tile([1, HALF], F32, tag="ast")
                nc.sync.dma_start(
                    out=ast[0:1, :].rearrange("o (p f) -> o p f", p=128),
                    in_=At,
                )

                # ---------- phase 3: w = Wc@xb, out = A*w (+ beta*xb) ----------
                for s in range(HALF // SUB):
                    gpix = half * HALF + s * SUB
                    c_ = gpix // CHUNK
                    off = gpix % CHUNK
                    sl = slice(off, off + SUB)
                    b_ = gpix // HWPIX
                    hw0 = gpix % HWPIX
                    ab = abp.tile([128, SUB], F32, tag="ab", name=f"ab{s}")
                    nc.gpsimd.partition_broadcast(
                        ab, ast[0:1, s * SUB : s * SUB + SUB]
                    )
                    wt = [None, None]
                    for mb in range(2):
                        wt[mb] = wps.tile([128, SUB], F32, tag=f"w{mb}", name=f"w{mb}_{s}")
                        for kb in range(2):
                            nc.tensor.matmul(
                                wt[mb], wm[kb][mb], xbt[kb][c_][:, sl],
                                start=(kb == 0), stop=(kb == 1),
                            )
                    ot = outp.tile([128, 2, SUB], F32, tag="ot")
                    for mb in range(2):
                        nc.vector.tensor_tensor(ot[:, mb, :], wt[mb], ab, AT.mult)
                        if beta_nonzero:
                            nc.vector.scalar_tensor_tensor(
                                ot[:, mb, :],
                                xbt[mb][c_][:, sl],
                                cvs(ID_BETA),
                                ot[:, mb, :],
                                AT.mult,
                                AT.add,
                            )
                    nc.sync.dma_start(
                        out=ov[:, :, b_, hw0 : hw0 + SUB].rearrange(
                            "cb p hw -> p cb hw"
                        ),
                        in_=ot,
                    )
    nc.finalize()
    return nc


def _chain_funcs(a0_1, a_1, b_1, a0_2, a_2, b_2):
    sc = np.sqrt(C_CURV)
    n = np.arange(1, N_HARM + 1)

    def fser(r, a0_, a, b):
        return (
            a0_
            + np.cos(np.outer(r, n)) @ np.asarray(a, np.float64)
            + np.sin(np.outer(r, n)) @ np.asarray(b, np.float64)
        )

    def g_of_r(r):
        rn = np.maximum(r, EPS)
        arg = np.minimum(sc * rn, 1 - 1e-5)
        return np.arctanh(arg) / (sc * rn)

    def P1f(y):
        r = np.sqrt(y)
        g = g_of_r(r)
        rn1 = np.maximum(np.abs(g) * r, EPS)
        return g * fser(rn1, a0_1, a_1, b_1)

    def Q2f(z):
        rn2 = np.maximum(np.sqrt(z), EPS)
        f2 = fser(rn2, a0_2, a_2, b_2)
        r3 = np.maximum(np.abs(f2) * rn2, EPS)
        return f2 * np.tanh(sc * r3) / (sc * r3)

    return P1f, Q2f


def _fit_poly(fn, lo, hi, tol):
    """Fit fn over [lo,hi]; return (deg, coefs highest-first f32, s, t)."""
    xs = np.linspace(lo, hi, 4001)
    vals = fn(xs)
    u = (2 * xs - (lo + hi)) / (hi - lo)
    best = None
    for deg in range(6, MAXDEG + 1):
        coef = np.polynomial.chebyshev.chebfit(u, vals, deg)
        pc = np.polynomial.chebyshev.cheb2poly(coef)[::-1].astype(np.float32)
        err = np.abs(np.polyval(pc.astype(np.float64), u) - vals).max()
        best = (deg, pc)
        if err < tol:
            break
    deg, pc = best
    s_ = np.float32(2.0 / (hi - lo))
    t_ = np.float32(-(lo + hi) / (hi - lo))
    return deg, pc, s_, t_


def _build_wmat(phi):
    phi = np.asarray(phi, np.float64)
    ang = L * phi
    hf = np.cos(ang) + 1j * np.sin(ang)
    eye = np.eye(C, dtype=np.float64)
    wrows = np.fft.irfft(
        np.fft.rfft(eye, axis=1) * hf[None, : C // 2 + 1], n=C, axis=1
    )
    wm = np.empty((2, 2, 128, 128), np.float64)
    for kb in range(2):
        for mb in range(2):
            wm[kb, mb] = wrows[
                128 * kb : 128 * kb + 128, 128 * mb : 128 * mb + 128
            ]
    return wm, wrows


_PROGRAM_CACHE: dict = {}


def prepare(inputs):
    """Build (nc, in_maps) for the SPMD run from full inputs."""
    import ml_dtypes

    x = np.ascontiguousarray(np.asarray(inputs["x"], dtype=np.float32))
    a0_1 = float(np.asarray(inputs["a0_1"]).reshape(-1)[0])
    a_1 = np.asarray(inputs["a_1"], np.float64)
    b_1 = np.asarray(inputs["b_1"], np.float64)
    a0_2 = float(np.asarray(inputs["a0_2"]).reshape(-1)[0])
    a_2 = np.asarray(inputs["a_2"], np.float64)
    b_2 = np.asarray(inputs["b_2"], np.float64)
    phi = np.asarray(inputs["phi"], np.float64)
    alpha = float(np.asarray(inputs["alpha"]).reshape(-1)[0])
    beta = float(np.asarray(inputs["beta"]).reshape(-1)[0])

    wm, wm_full = _build_wmat(phi)
    P1f, Q2f = _chain_funcs(a0_1, a_1, b_1, a0_2, a_2, b_2)

    cos0 = np.cos(L * phi[0])
    cos128 = np.cos(L * phi[128])
    nd0 = -(1.0 - cos0 * cos0) / C
    nd1 = -(1.0 - cos128 * cos128) / C

    # data-dependent tight domains (x is available at prep time)
    xr64 = x.reshape(B, C, HWPIX).astype(np.float64)
    r2 = np.einsum("bcp,bcp->bp", xr64, xr64)
    X0 = xr64.sum(axis=1)
    altv = 1.0 - 2.0 * (np.arange(C) % 2)
    X128 = np.einsum("bcp,c->bp", xr64, altv)
    q2 = r2 + nd0 * X0 * X0 + nd1 * X128 * X128
    ylo = float(r2.min()) * 0.98
    yhi = float(r2.max()) * 1.02
    p1px = P1f(r2.reshape(-1))
    z_corr = p1px * p1px * q2.reshape(-1)
    z_unc = p1px * p1px * r2.reshape(-1)
    zlo = float(min(z_corr.min(), z_unc.min())) * 0.98
    zhi = float(max(z_corr.max(), z_unc.max())) * 1.02

    # can the DC/Nyquist correction be skipped? exact host-side bound on the
    # output error |A_err * w| using the true w at the affected pixels
    aerr = np.abs(p1px * alpha * (Q2f(z_corr) - Q2f(z_unc)))
    sel = aerr > 2.5e-4
    use_x = False
    if sel.any():
        xcols = xr64.transpose(0, 2, 1).reshape(-1, C)[sel]  # [n, 256]
        wsel = xcols @ wm_full  # v0_row = u1_row @ Wrows convention
        bound = float((aerr[sel] * np.abs(wsel).max(axis=1)).max())
        use_x = bound > 2.5e-3

    d1, p1c, s1, t1 = _fit_poly(P1f, ylo, yhi, 2e-4)
    d2, q2c, s2, t2 = _fit_poly(
        lambda z: alpha * Q2f(z), zlo, zhi, 2e-4 * max(abs(alpha), 1.0)
    )

    cvec = np.zeros((1, CV), np.float32)
    cvec[0, ID_P1 : ID_P1 + d1 + 1] = p1c
    cvec[0, ID_Q2 : ID_Q2 + d2 + 1] = q2c
    cvec[0, ID_S1] = s1
    cvec[0, ID_T1] = t1
    cvec[0, ID_S2] = s2
    cvec[0, ID_T2] = t2
    cvec[0, ID_ND0] = nd0
    cvec[0, ID_ND1] = nd1
    cvec[0, ID_YLO] = ylo
    cvec[0, ID_YHI] = yhi
    cvec[0, ID_ZLO] = zlo
    cvec[0, ID_ZHI] = zhi
    cvec[0, ID_BETA] = beta

    saltv = np.zeros((128, 32), np.float32)
    saltv[:, 0] = 1.0
    saltv[:, 1] = 1.0 - 2.0 * (np.arange(128) % 2)

    beta_nonzero = beta != 0.0
    key = (beta_nonzero, use_x, d1, d2)
    if key not in _PROGRAM_CACHE:
        _PROGRAM_CACHE[key] = build_program(beta_nonzero, use_x, d1, d2)
    nc = _PROGRAM_CACHE[key]

    wm_bf = wm.astype(ml_dtypes.bfloat16)
    salt_bf = saltv.astype(ml_dtypes.bfloat16)
    xr = x.reshape(B, C, HWPIX)
    in_maps = []
    for k in range(NCORES):
        in_maps.append(
            {
                "x": xr[k * B_CORE : (k + 1) * B_CORE],
                "wmat": wm_bf,
                "saltv": salt_bf,
                "cvec": cvec,
            }
        )
    return nc, in_maps


def kernel(**inputs) -> np.ndarray:
    nc, in_maps = prepare(inputs)

    from concourse.bass_utils import run_bass_kernel_spmd

    res = run_bass_kernel_spmd(nc, in_maps, list(range(NCORES)))
    out = np.concatenate([np.asarray(r["out"]) for r in res.results], axis=0)
    return out.reshape(B, C, H, W)


# revision 17
# speedup vs baseline: 1.2851x; 1.2851x over previous
"""Trainium2 Bass kernel for nn_ChannelNonlinearSpectralBlock.

Math
----
Per pixel column x (C=256 channels), the reference computes
    u  = g(||x||) * x                      (log map, per-pixel scalar gate)
    u1 = f1(||u||) * u                     (Fourier gate 1)
    v0 = irfft(rfft(u1) * Hf)              (fixed linear map: circulant Wc)
    v1 = f2(||v0||) * v0                   (Fourier gate 2)
    y  = t(||v1||) * v1                    (exp map)
    out = alpha*y + beta*x
All per-pixel scalars commute through the linear map Wc, so
    out = A * (Wc @ x) + beta * x
with A = alpha * P1(r0^2) * Q2(P1(r0^2)^2 * q^2), r0^2 = ||x||^2 and (by
Parseval; |Hf|=1 except the real-projected DC/Nyquist bins)
    q^2 = r0^2 - d0*X0^2 - d1*X128^2,
    X0 = sum_c x[c], X128 = sum_c (-1)^c x[c].
P1, Q2 are fitted host-side as polynomials over the observed data range
(coefficients are runtime data via a constant vector).  When the exactly
computed host-side bound on the d0/d1 correction's effect on A is far below
tolerance, the X0/X128 stats are skipped entirely (use_x=False variant).

Implementation (per core, pure data parallel over pixels)
---------------------------------------------------------
Channel-major layout [128 chans x pixels].  ScalarE casts x to bf16 once;
VectorE/GpSimd square the bf16 copy; TensorE reduces r0^2 with a resident
ones-column bf16 stationary (PSUM rows packed at {0,32,64} offsets) and
computes the 256x256 circulant matmul in bf16 (2x stream rate vs f32r).
Stats evacuate via strided engine copies + one rearrange DMA per chunk.
The gate chain runs on VectorE with scalar_tensor_tensor fused Horner
steps (1 op per degree).  GpSimd broadcasts A across partitions; VectorE
applies A * (Wc x) out of PSUM; outputs stream back with 0.5 MB DMAs.
"""

import numpy as np

import concourse.bass as bass
import concourse.bacc as bacc
from concourse import library_config
import concourse.mybir as mybir
from concourse.tile import TileContext

F32 = mybir.dt.float32
F32R = mybir.dt.float32r
BF16 = mybir.dt.bfloat16

# Problem shape (hardcoded per contract)
B, C, H, W = 32, 256, 64, 64
HWPIX = H * W  # 4096
NCORES = 8
B_CORE = B // NCORES  # 4 images per core
NPIX = B_CORE * HWPIX  # 16384 pixels per core
HALF = NPIX // 2  # 8192
CHUNK = 2048
N_CHUNK = NPIX // CHUNK  # 8
CH_HALF = N_CHUNK // 2  # 4 chunks per half
SUB = 512
N_SUB_CH = CHUNK // SUB  # 4 subtiles per chunk

C_CURV = 0.001
L = 10
N_HARM = 16
EPS = 1e-6

MAXDEG = 20

# cvec layout: [P1 coefs (MAXDEG+1, highest first)][Q2 coefs][misc]
ID_P1 = 0
ID_Q2 = MAXDEG + 1
ID_S1 = 2 * (MAXDEG + 1)
ID_T1 = ID_S1 + 1
ID_S2 = ID_S1 + 2
ID_T2 = ID_S1 + 3
ID_ND0 = ID_S1 + 4
ID_ND1 = ID_S1 + 5
ID_YLO = ID_S1 + 6
ID_YHI = ID_S1 + 7
ID_ZLO = ID_S1 + 8
ID_ZHI = ID_S1 + 9
ID_BETA = ID_S1 + 10
CV = 64
assert ID_BETA < CV


def build_program(beta_nonzero: bool, use_x: bool, d1: int, d2: int) -> bass.Bass:
    nc = bacc.Bacc(None, target_bir_lowering=False)
    x = nc.declare_dram_parameter("x", [B_CORE, C, HWPIX], F32, isOutput=False)
    wmat = nc.declare_dram_parameter("wmat", [2, 2, 128, 128], BF16, isOutput=False)
    saltv = nc.declare_dram_parameter("saltv", [128, 32], BF16, isOutput=False)
    cvec = nc.declare_dram_parameter("cvec", [1, CV], F32, isOutput=False)
    out = nc.declare_dram_parameter("out", [B_CORE, C, HWPIX], F32, isOutput=True)

    # channel-block views of DRAM x / out: [cb, 128, b, hw]
    xv = x.rearrange("b (cb p) hw -> cb p b hw", cb=2)
    ov = out.rearrange("b (cb p) hw -> cb p b hw", cb=2)

    AT = mybir.AluOpType

    with TileContext(nc) as tc:
        with (
            tc.tile_pool(name="const", bufs=1) as const,
            tc.tile_pool(name="xf", bufs=2) as xfp,
            tc.tile_pool(name="xb", bufs=1) as xbp,
            tc.tile_pool(name="sq", bufs=2) as sqp,
            tc.tile_pool(name="stg", bufs=2) as stgp,
            tc.tile_pool(name="chain", bufs=1) as chp,
            tc.tile_pool(name="astage", bufs=1) as asp,
            tc.tile_pool(name="abp", bufs=3) as abp,
            tc.tile_pool(name="outp", bufs=3) as outp,
            tc.tile_pool(name="stps", bufs=1 if use_x else 2, space="PSUM") as stps,
            tc.tile_pool(name="wps", bufs=3, space="PSUM") as wps,
        ):
            nc.gpsimd.load_library(library_config.mlp)

            # ---- constants ----
            wm = [[None, None], [None, None]]
            for kb in range(2):
                for mb in range(2):
                    t = const.tile([128, 128], BF16, name=f"wm{kb}{mb}", tag=f"wm{kb}{mb}")
                    nc.sync.dma_start(out=t, in_=wmat[kb, mb])
                    wm[kb][mb] = t
            salt = const.tile([128, 32], BF16, tag="saltv")
            nc.sync.dma_start(out=salt, in_=saltv[:, :])
            cv = const.tile([128, CV], F32, tag="cv")
            nc.sync.dma_start(out=cv, in_=cvec[0:1, :].partition_broadcast(128))

            def cvs(i):  # per-partition scalar AP for constant i
                return cv[:, i : i + 1]

            # resident bf16 x (both channel blocks, all chunks)
            xbt = [[None] * N_CHUNK, [None] * N_CHUNK]
            for blk in range(2):
                for c_ in range(N_CHUNK):
                    xbt[blk][c_] = xbp.tile(
                        [128, CHUNK], BF16, name=f"xb{blk}_{c_}", tag=f"xb{blk}_{c_}"
                    )

            chtiles = {}

            def emit_phase1(half):
                chR = chp.tile([128, 64], F32, tag=f"chR{half}")
                chS0 = chS1 = None
                if use_x:
                    chS0 = chp.tile([128, 64], F32, tag=f"chS0{half}")
                    chS1 = chp.tile([128, 64], F32, tag=f"chS1{half}")

                for t_ in range(CH_HALF):
                    c_ = half * CH_HALF + t_
                    b_ = (c_ * CHUNK) // HWPIX
                    hw0 = (c_ * CHUNK) % HWPIX
                    sq = [None, None]
                    tsum = None
                    for blk in range(2):
                        xf = xfp.tile([128, CHUNK], F32, name=f"xf{blk}", tag=f"xf{blk}")
                        nc.sync.dma_start(
                            out=xf, in_=xv[blk, :, b_, hw0 : hw0 + CHUNK]
                        )
                        # cast to bf16 (ScalarE)
                        nc.scalar.copy(xbt[blk][c_], xf)
                        sq[blk] = sqp.tile(
                            [128, CHUNK], BF16, name=f"sq{blk}", tag=f"sq{blk}"
                        )
                        if blk == 0:
                            # ScalarE squares block 0 from the f32 tile
                            nc.scalar.activation(
                                sq[blk], xf,
                                mybir.ActivationFunctionType.Square,
                            )
                        else:
                            # VectorE squares block 1 (bf16 in/out, 2x mode)
                            nc.vector.tensor_tensor(
                                sq[blk], xbt[blk][c_], xbt[blk][c_], AT.mult
                            )
                    if use_x:
                        tsum = sqp.tile([128, CHUNK], BF16, tag="tsum")
                        nc.vector.tensor_tensor(
                            tsum, xbt[0][c_], xbt[1][c_], AT.add
                        )
                    # r0^2: subtile s4 row at PSUM partition 32*s4 (one bank)
                    stR = stps.tile([128, SUB], F32, tag="stR")
                    if use_x:
                        stX = stps.tile([128, SUB], F32, tag="stX")
                    for s4 in range(N_SUB_CH):
                        sl = slice(s4 * SUB, s4 * SUB + SUB)
                        rout = stR[32 * s4 : 32 * s4 + 32, :]
                        nc.tensor.matmul(
                            rout, salt, sq[0][:, sl],
                            start=True, stop=False, tile_position=(0, 32 * s4),
                        )
                        nc.tensor.matmul(
                            rout, salt, sq[1][:, sl],
                            start=False, stop=True, tile_position=(0, 32 * s4),
                        )
                        if use_x:
                            nc.tensor.matmul(
                                stX[32 * s4 : 32 * s4 + 32, :], salt,
                                tsum[:, sl],
                                start=True, stop=True,
                                tile_position=(0, 32 * s4),
                            )
                    # evacuate: one contiguous engine copy PSUM->SBUF, then a
                    # partition-strided gather DMA into the chain layout
                    rstg = stgp.tile([128, SUB], F32, tag="rstg")
                    nc.scalar.copy(rstg, stR)
                    p0 = 32 * t_
                    nc.sync.dma_start(
                        out=chR[p0 : p0 + 32, :], in_=rstg[0:128:32, :]
                    )
                    if use_x:
                        xstg = stgp.tile([128, SUB], F32, tag="xstg")
                        nc.scalar.copy(xstg, stX)
                        nc.sync.dma_start(
                            out=chS0[p0 : p0 + 32, :], in_=xstg[0:128:32, :]
                        )
                        for s4 in range(N_SUB_CH):
                            nc.sync.dma_start(
                                out=chS1[p0 + 8 * s4 : p0 + 8 * s4 + 8, :],
                                in_=xstg[
                                    32 * s4 + 1 : 32 * s4 + 2, :
                                ].rearrange("o (p f) -> o p f", p=8),
                            )
                chtiles[half] = (chR, chS0, chS1)

            def emit_chain(half):
                chR, chS0, chS1 = chtiles[half]
                yc = chp.tile([128, 64], F32, tag="yc")
                q2 = chp.tile([128, 64], F32, tag="q2")
                ut = chp.tile([128, 64], F32, tag="ut")
                acc = chp.tile([128, 64], F32, tag="acc")
                p1 = chp.tile([128, 64], F32, tag="p1")
                zt = chp.tile([128, 64], F32, tag="zt")
                acc2 = chp.tile([128, 64], F32, tag="acc2")
                At = chp.tile([128, 64], F32, tag="At")

                V = nc.vector
                if use_x:
                    t0 = chp.tile([128, 64], F32, tag="t0")
                    # q2 = chR + nd0*X0^2 + nd1*X128^2 (nd* negated host-side)
                    V.scalar_tensor_tensor(t0, chS0, cvs(ID_ND0), chS0, AT.mult, AT.mult)
                    V.scalar_tensor_tensor(q2, chS1, cvs(ID_ND1), chS1, AT.mult, AT.mult)
                    V.tensor_tensor(q2, q2, t0, AT.add)
                    V.tensor_tensor(q2, q2, chR, AT.add)

                # u = clamp(r0^2)*s1 + t1 in [-1,1]
                V.tensor_scalar(yc, chR, cvs(ID_YLO), cvs(ID_YHI), AT.max, AT.min)
                V.tensor_scalar(ut, yc, cvs(ID_S1), cvs(ID_T1), AT.mult, AT.add)
                # P1 modified Horner: b=c_D*u; b=(b+c_k)*u ...; p1=b+c_0
                V.tensor_scalar(acc, ut, cvs(ID_P1), None, AT.mult)
                for k in range(1, d1):
                    V.scalar_tensor_tensor(
                        acc, acc, cvs(ID_P1 + k), ut, AT.add, AT.mult
                    )
                V.tensor_scalar(p1, acc, cvs(ID_P1 + d1), None, AT.add)

                # z = p1^2 * q2 (q2 = clamped r0^2 when X-stats skipped)
                V.tensor_tensor(zt, p1, p1, AT.mult)
                V.tensor_tensor(zt, zt, q2 if use_x else yc, AT.mult)
                V.tensor_scalar(zt, zt, cvs(ID_ZLO), cvs(ID_ZHI), AT.max, AT.min)
                V.tensor_scalar(zt, zt, cvs(ID_S2), cvs(ID_T2), AT.mult, AT.add)
                # Q2 modified Horner (alpha folded into coefficients)
                V.tensor_scalar(acc2, zt, cvs(ID_Q2), None, AT.mult)
                for k in range(1, d2):
                    V.scalar_tensor_tensor(
                        acc2, acc2, cvs(ID_Q2 + k), zt, AT.add, AT.mult
                    )
                V.tensor_scalar(acc2, acc2, cvs(ID_Q2 + d2), None, AT.add)

                V.tensor_tensor(At, p1, acc2, AT.mult)

                ast = asp.tile([1, HALF], F32, tag="ast", name=f"ast{half}")
                nc.sync.dma_start(
                    out=ast[0:1, :].rearrange("o (p f) -> o p f", p=128),
                    in_=At,
                )
                return ast

            def emit_phase3(half, ast):
                for s in range(HALF // SUB):
                    gpix = half * HALF + s * SUB
                    c_ = gpix // CHUNK
                    off = gpix % CHUNK
                    sl = slice(off, off + SUB)
                    b_ = gpix // HWPIX
                    hw0 = gpix % HWPIX
                    ab = abp.tile([128, SUB], F32, tag="ab", name=f"ab{s}")
                    nc.gpsimd.partition_broadcast(
                        ab, ast[0:1, s * SUB : s * SUB + SUB]
                    )
                    wt = [None, None]
                    for mb in range(2):
                        wt[mb] = wps.tile([128, SUB], F32, tag=f"w{mb}", name=f"w{mb}_{s}")
                        for kb in range(2):
                            nc.tensor.matmul(
                                wt[mb], wm[kb][mb], xbt[kb][c_][:, sl],
                                start=(kb == 0), stop=(kb == 1),
                            )
                    ot = outp.tile([128, 2, SUB], F32, tag="ot")
                    for mb in range(2):
                        nc.vector.tensor_tensor(ot[:, mb, :], wt[mb], ab, AT.mult)
                        if beta_nonzero:
                            nc.vector.scalar_tensor_tensor(
                                ot[:, mb, :],
                                xbt[mb][c_][:, sl],
                                cvs(ID_BETA),
                                ot[:, mb, :],
                                AT.mult,
                                AT.add,
                            )
                    nc.sync.dma_start(
                        out=ov[:, :, b_, hw0 : hw0 + SUB].rearrange(
                            "cb p hw -> p cb hw"
                        ),
                        in_=ot,
                    )

            # emission order keeps DMA-in ahead of DMA-out and lets PE run
            # half-1 stats while half-0 phase 3 waits on the gate chain
            emit_phase1(0)
            ast0 = emit_chain(0)
            emit_phase1(1)
            emit_phase3(0, ast0)
            ast1 = emit_chain(1)
            emit_phase3(1, ast1)
    nc.finalize()
    return nc


def _chain_funcs(a0_1, a_1, b_1, a0_2, a_2, b_2):
    sc = np.sqrt(C_CURV)
    n = np.arange(1, N_HARM + 1)

    def fser(r, a0_, a, b):
        return (
            a0_
            + np.cos(np.outer(r, n)) @ np.asarray(a, np.float64)
            + np.sin(np.outer(r, n)) @ np.asarray(b, np.float64)
        )

    def g_of_r(r):
        rn = np.maximum(r, EPS)
        arg = np.minimum(sc * rn, 1 - 1e-5)
        return np.arctanh(arg) / (sc * rn)

    def P1f(y):
        r = np.sqrt(y)
        g = g_of_r(r)
        rn1 = np.maximum(np.abs(g) * r, EPS)
        return g * fser(rn1, a0_1, a_1, b_1)

    def Q2f(z):
        rn2 = np.maximum(np.sqrt(z), EPS)
        f2 = fser(rn2, a0_2, a_2, b_2)
        r3 = np.maximum(np.abs(f2) * rn2, EPS)
        return f2 * np.tanh(sc * r3) / (sc * r3)

    return P1f, Q2f


def _fit_poly(fn, lo, hi, tol):
    """Fit fn over [lo,hi]; return (deg, coefs highest-first f32, s, t)."""
    xs = np.linspace(lo, hi, 4001)
    vals = fn(xs)
    u = (2 * xs - (lo + hi)) / (hi - lo)
    best = None
    for deg in range(6, MAXDEG + 1):
        coef = np.polynomial.chebyshev.chebfit(u, vals, deg)
        pc = np.polynomial.chebyshev.cheb2poly(coef)[::-1].astype(np.float32)
        err = np.abs(np.polyval(pc.astype(np.float64), u) - vals).max()
        best = (deg, pc)
        if err < tol:
            break
    deg, pc = best
    s_ = np.float32(2.0 / (hi - lo))
    t_ = np.float32(-(lo + hi) / (hi - lo))
    return deg, pc, s_, t_


def _build_wmat(phi):
    phi = np.asarray(phi, np.float64)
    ang = L * phi
    hf = np.cos(ang) + 1j * np.sin(ang)
    eye = np.eye(C, dtype=np.float64)
    wrows = np.fft.irfft(
        np.fft.rfft(eye, axis=1) * hf[None, : C // 2 + 1], n=C, axis=1
    )
    wm = np.empty((2, 2, 128, 128), np.float64)
    for kb in range(2):
        for mb in range(2):
            wm[kb, mb] = wrows[
                128 * kb : 128 * kb + 128, 128 * mb : 128 * mb + 128
            ]
    return wm, wrows


_PROGRAM_CACHE: dict = {}


def prepare(inputs):
    """Build (nc, in_maps) for the SPMD run from full inputs."""
    import ml_dtypes

    x = np.ascontiguousarray(np.asarray(inputs["x"], dtype=np.float32))
    a0_1 = float(np.asarray(inputs["a0_1"]).reshape(-1)[0])
    a_1 = np.asarray(inputs["a_1"], np.float64)
    b_1 = np.asarray(inputs["b_1"], np.float64)
    a0_2 = float(np.asarray(inputs["a0_2"]).reshape(-1)[0])
    a_2 = np.asarray(inputs["a_2"], np.float64)
    b_2 = np.asarray(inputs["b_2"], np.float64)
    phi = np.asarray(inputs["phi"], np.float64)
    alpha = float(np.asarray(inputs["alpha"]).reshape(-1)[0])
    beta = float(np.asarray(inputs["beta"]).reshape(-1)[0])

    wm, wm_full = _build_wmat(phi)
    P1f, Q2f = _chain_funcs(a0_1, a_1, b_1, a0_2, a_2, b_2)

    cos0 = np.cos(L * phi[0])
    cos128 = np.cos(L * phi[128])
    nd0 = -(1.0 - cos0 * cos0) / C
    nd1 = -(1.0 - cos128 * cos128) / C

    # data-dependent tight domains (x is available at prep time)
    xr64 = x.reshape(B, C, HWPIX).astype(np.float64)
    r2 = np.einsum("bcp,bcp->bp", xr64, xr64)
    X0 = xr64.sum(axis=1)
    altv = 1.0 - 2.0 * (np.arange(C) % 2)
    X128 = np.einsum("bcp,c->bp", xr64, altv)
    q2 = r2 + nd0 * X0 * X0 + nd1 * X128 * X128
    ylo = float(r2.min()) * 0.98
    yhi = float(r2.max()) * 1.02
    p1px = P1f(r2.reshape(-1))
    z_corr = p1px * p1px * q2.reshape(-1)
    z_unc = p1px * p1px * r2.reshape(-1)
    zlo = float(min(z_corr.min(), z_unc.min())) * 0.98
    zhi = float(max(z_corr.max(), z_unc.max())) * 1.02

    # can the DC/Nyquist correction be skipped? exact host-side bound on the
    # output error |A_err * w| using the true w at the affected pixels
    aerr = np.abs(p1px * alpha * (Q2f(z_corr) - Q2f(z_unc)))
    sel = aerr > 2.5e-4
    use_x = False
    if sel.any():
        xcols = xr64.transpose(0, 2, 1).reshape(-1, C)[sel]  # [n, 256]
        wsel = xcols @ wm_full  # v0_row = u1_row @ Wrows convention
        bound = float((aerr[sel] * np.abs(wsel).max(axis=1)).max())
        use_x = bound > 2.5e-3

    d1, p1c, s1, t1 = _fit_poly(P1f, ylo, yhi, 4e-4)
    d2, q2c, s2, t2 = _fit_poly(
        lambda z: alpha * Q2f(z), zlo, zhi, 4e-4 * max(abs(alpha), 1.0)
    )

    cvec = np.zeros((1, CV), np.float32)
    cvec[0, ID_P1 : ID_P1 + d1 + 1] = p1c
    cvec[0, ID_Q2 : ID_Q2 + d2 + 1] = q2c
    cvec[0, ID_S1] = s1
    cvec[0, ID_T1] = t1
    cvec[0, ID_S2] = s2
    cvec[0, ID_T2] = t2
    cvec[0, ID_ND0] = nd0
    cvec[0, ID_ND1] = nd1
    cvec[0, ID_YLO] = ylo
    cvec[0, ID_YHI] = yhi
    cvec[0, ID_ZLO] = zlo
    cvec[0, ID_ZHI] = zhi
    cvec[0, ID_BETA] = beta

    saltv = np.zeros((128, 32), np.float32)
    saltv[:, 0] = 1.0
    saltv[:, 1] = 1.0 - 2.0 * (np.arange(128) % 2)

    beta_nonzero = beta != 0.0
    key = (beta_nonzero, use_x, d1, d2)
    if key not in _PROGRAM_CACHE:
        _PROGRAM_CACHE[key] = build_program(beta_nonzero, use_x, d1, d2)
    nc = _PROGRAM_CACHE[key]

    wm_bf = wm.astype(ml_dtypes.bfloat16)
    salt_bf = saltv.astype(ml_dtypes.bfloat16)
    xr = x.reshape(B, C, HWPIX)
    in_maps = []
    for k in range(NCORES):
        in_maps.append(
            {
                "x": xr[k * B_CORE : (k + 1) * B_CORE],
                "wmat": wm_bf,
                "saltv": salt_bf,
                "cvec": cvec,
            }
        )
    return nc, in_maps


def kernel(**inputs) -> np.ndarray:
    nc, in_maps = prepare(inputs)

    from concourse.bass_utils import run_bass_kernel_spmd

    res = run_bass_kernel_spmd(nc, in_maps, list(range(NCORES)))
    out = np.concatenate([np.asarray(r["out"]) for r in res.results], axis=0)
    return out.reshape(B, C, H, W)
